# revision 13
# baseline (speedup 1.0000x reference)
"""Trainium2 Bass kernel for nn_CPWGenerator (B=16384, D=128, P=10, F=1024).

Data-parallel over batch across 8 NeuronCores (2048 rows/core). Per core:
  - feature-major 3-layer MLPs (control-point head + weight head)
  - softmax denominator cancels: out = (basis @ (e*cpm)) / (basis @ e)
  - KEY RESTRUCTURE vs the first version: the ratio num/den is evaluated at
    only S=64 coarse t-samples (basis row-normalization cancels in the
    ratio), then upsampled to F=1024 by a single PE matmul against a
    precomputed cubic-Lagrange interpolation matrix im [S, F]. The ratio is
    a sum of 10 Gaussians with sigma ~ 102 grid points, so cubic
    interpolation from 64 uniform samples adds < 2e-4 relative error
    (measured end-to-end: 5.2e-4 total vs reference, gate is 2e-2).
    This cuts the full-resolution elementwise work (reciprocal + multiply
    at [128, 1024] per tile) down to one divide at [64, 512] per block.
  - PSUM->SBUF evictions are spread across ACT, DVE and GPSIMD; output DMA
    is spread across the SP, ACT and GPSIMD queues.
Matmuls run as float32r (fp32 storage, 11-bit-mantissa operand rounding,
exact fp32 accumulation) at full PE rate.
"""
import sys
if "/opt/trn_rl_repo" not in sys.path:
    sys.path.insert(0, "/opt/trn_rl_repo")

from contextlib import ExitStack

import numpy as np

import concourse.bacc as bacc
import concourse.mybir as mybir
import concourse.tile as tile
from concourse.bass_utils import run_bass_kernel_spmd

F32 = mybir.dt.float32
F32R = mybir.dt.float32r
AF = mybir.ActivationFunctionType
ALU = mybir.AluOpType

# problem shapes (hardcoded per contest contract)
B, D, P, F = 16384, 128, 10, 1024
NCORES = 8
BC = B // NCORES          # rows per core = 2048
NB = 512                  # batch block
NBLK = BC // NB           # 4
S = 64                    # coarse t-samples for the ratio
EPS = 1e-8

# f32r const blob (wr) column offsets
_C_W1T = 0            # [128 x 128]
_C_W2T = 128          # [128 x 256]
_C_W3T = 384          # [128 x 40]  (W3Ta | W3Tb, 20 cols each)
_C_WW1T = 424         # [128 x 64]
_C_WW2T = 488         # [64  x 128]
_C_WW3T = 616         # [128 x 10]
_C_P20 = 626          # [20  x 10]  pairing matrix (0.5 per pair)
_C_BTC = 636          # [10  x 64]  unnormalized basis at the S t-samples
_C_IDR = 700          # [128 x 128] identity (for PE transpose)
C_R = 828

# fp32 bias blob (wf) columns
_C_B1 = 0
_C_B2A = 1
_C_B2B = 2
_C_B3 = 3             # rows 0-19
_C_WB1 = 4            # rows 0-63
_C_WB2 = 5
_C_WB3 = 6            # rows 0-9
C_F = 7

# ---- engine assignment tables (tunable) ----
# (GPSIMD cannot access PSUM — BIR verifier — so evictions are ACT/DVE only.)
# MLP eviction engine per (layer, block parity): 'A' = ACT, 'V' = DVE.
# Pair-lockstep runs blocks (2k, 2k+1) interleaved; same layer of the two
# blocks goes to opposite engines so the evicts run concurrently.
EVP = {"xt": "AV", "h1": "VA", "h2a": "AV", "h2b": "VA",
       "g1": "AV", "g2": "VA"}
# out-stage eviction engines per j-tile half (cols 0:512, 512:1024)
OEV = [("A", "V"), ("V", "A")] * 8
# out DMA queue per j-tile: 'S' = SP, 'A' = ACT, 'P' = GPSIMD
OQ = list("SPSS" "SPSP" "SPSS" "SPSP")
TICK_EVERY = 4            # stage-yields between injected out-tiles

MLP_BUFS = 2
SM_BUFS = 2
UP_BUFS = 4


def round_f32r(x: np.ndarray) -> np.ndarray:
    """fp32 -> fp32r rounding (keep 11 explicit mantissa bits, RNE)."""
    u = np.ascontiguousarray(x, dtype=np.float32).view(np.uint32)
    keep = np.uint32(0xFFFFF000)
    half = np.uint32(0x800)
    lsb = (u >> np.uint32(12)) & np.uint32(1)
    r = (u + half - np.uint32(1) + lsb) & keep
    return r.view(np.float32)


def basis_coarse() -> np.ndarray:
    """Unnormalized Gaussian basis at S uniform t-samples: [P, S].
    Row-normalization of the reference basis cancels in num/den."""
    t = np.linspace(0.0, 1.0, S, dtype=np.float64)
    centers = np.arange(P, dtype=np.float64) / (P - 1)
    sigma = 1.0 / P
    z = t[None, :] - centers[:, None]
    return np.exp(-(z * z) / (2.0 * sigma * sigma)).astype(np.float32)


def interp_matrix() -> np.ndarray:
    """Cubic Lagrange interpolation weights from S uniform t-samples to the
    F output t-points: [S, F]."""
    t_s = np.linspace(0.0, 1.0, S)
    t_f = np.linspace(0.0, 1.0, F)
    I = np.zeros((S, F), np.float64)
    for fi, t in enumerate(t_f):
        k = np.searchsorted(t_s, t) - 1
        k = int(np.clip(k, 1, S - 3))
        idx = [k - 1, k, k + 1, k + 2]
        for a in range(4):
            w = 1.0
            for bb in range(4):
                if a != bb:
                    w *= (t - t_s[idx[bb]]) / (t_s[idx[a]] - t_s[idx[bb]])
            I[idx[a], fi] += w
    return I.astype(np.float32)


def build_program():
    nc = bacc.Bacc()
    x_in = nc.declare_dram_parameter("x", [BC, D], F32R, isOutput=False)
    wr_in = nc.declare_dram_parameter("wr", [128, C_R], F32R, isOutput=False)
    wf_in = nc.declare_dram_parameter("wf", [128, C_F], F32, isOutput=False)
    im_in = nc.declare_dram_parameter("im", [S, F], F32R, isOutput=False)
    out = nc.declare_dram_parameter("out", [BC, F], F32, isOutput=True)

    with tile.TileContext(nc) as tc, ExitStack() as ctx:
        cpool = ctx.enter_context(tc.tile_pool(name="const", bufs=1))
        wpool = ctx.enter_context(tc.tile_pool(name="work", bufs=2))
        opool = ctx.enter_context(tc.tile_pool(name="outp", bufs=4))
        ppool = ctx.enter_context(tc.tile_pool(name="ps", bufs=1, space="PSUM"))

        wr = cpool.tile([128, C_R], F32R)
        wf = cpool.tile([128, C_F], F32)
        im = cpool.tile([S, F], F32R)
        xall = cpool.tile([128, BC], F32R)

        def x_dma(eng, r0, r1):
            eng.dma_start(
                xall[:, r0:r1].rearrange("p (c d) -> p c d", c=(r1 - r0) // 128),
                x_in[r0:r1, :].rearrange("(c p) d -> p c d", p=128),
            )

        # fill-phase DMAs, ordered by first use; ACT stays DMA-free (it is an
        # eviction bottleneck engine). SP: ident -> x blocks -> interp matrix.
        # Pool: biases -> weights -> x blocks.
        x_dma(nc.sync, 0, 512)
        nc.gpsimd.dma_start(wr[:, _C_IDR:_C_IDR + 128],
                            wr_in[:, _C_IDR:_C_IDR + 128])
        nc.gpsimd.dma_start(wf[:], wf_in[:])
        nc.gpsimd.dma_start(wr[:, 0:_C_IDR], wr_in[:, 0:_C_IDR])
        x_dma(nc.sync, 512, 1024)
        x_dma(nc.sync, 1024, 1536)
        nc.sync.dma_start(im[:], im_in[:])
        x_dma(nc.gpsimd, 1536, 2048)

        def mm(out_ap, lhsT, rhs, start=True, stop=True):
            nc.tensor.matmul(out_ap, lhsT, rhs, start=start, stop=stop)

        def evict(eng, out_ap, in_ap, bias=None, act=None):
            """PSUM -> SBUF eviction with optional bias+activation."""
            if eng == "A":
                if act is None:
                    nc.scalar.activation(out_ap, in_ap, AF.Copy)
                else:
                    nc.scalar.activation(out_ap, in_ap, act, bias=bias)
            else:
                e = nc.vector if eng == "V" else nc.gpsimd
                if act is None and bias is None:
                    e.tensor_scalar(out=out_ap, in0=in_ap, scalar1=0.0,
                                    scalar2=None, op0=ALU.add)
                elif act == AF.Relu:
                    e.tensor_scalar(out=out_ap, in0=in_ap, scalar1=bias,
                                    scalar2=0.0, op0=ALU.add, op1=ALU.max)
                else:
                    raise ValueError((eng, act))

        ratios = [None] * NBLK
        pending = []          # j-tiles awaiting emission (ready after ratio)

        def emit_j(b, jj):
            x0 = NB * b
            ratio = ratios[b]
            j = (NB // 128) * b + jj
            lhsT = ratio[:, 128 * jj:128 * (jj + 1)]
            upa = ppool.tile([128, 512], F32, tag="up", bufs=UP_BUFS,
                             name=f"upa{j}")
            mm(upa[:], lhsT, im[:, 0:512])
            upb = ppool.tile([128, 512], F32, tag="up", bufs=UP_BUFS,
                             name=f"upb{j}")
            mm(upb[:], lhsT, im[:, 512:1024])
            o = opool.tile([128, F], F32, tag="o", name=f"o{j}")
            evict(OEV[j][0], o[:, 0:512], upa[:])
            evict(OEV[j][1], o[:, 512:1024], upb[:])
            q = {"S": nc.sync, "A": nc.scalar, "P": nc.gpsimd}[OQ[j]]
            q.dma_start(out[x0 + 128 * jj:x0 + 128 * (jj + 1), :], o[:])

        def tick():
            if pending:
                emit_j(*pending.pop(0))

        def stages(b):
            """Generator emitting one block's ops, yielding between stages so
            the driver can interleave the pair partner + out-tiles."""
            par = b % 2
            x0 = NB * b

            xtp = ppool.tile([128, NB], F32R, tag="mlp", bufs=MLP_BUFS,
                             name=f"xtp{b}")
            for c in range(NB // 128):
                nc.tensor.matmul(
                    xtp[:, 128 * c:128 * (c + 1)],
                    xall[:, x0 + 128 * c:x0 + 128 * (c + 1)],
                    wr[:, _C_IDR:_C_IDR + 128],
                    is_transpose=True,
                    start=(c == 0), stop=(c == NB // 128 - 1),
                )
            yield
            xt = wpool.tile([128, NB], F32R, tag="xt", bufs=2, name=f"xt{b}")
            evict(EVP["xt"][par], xt[:], xtp[:].bitcast(F32))
            yield

            h1p = ppool.tile([128, NB], F32, tag="mlp", bufs=MLP_BUFS,
                             name=f"h1p{b}")
            mm(h1p[:], wr[:, _C_W1T:_C_W1T + 128], xt[:])
            yield
            h1 = wpool.tile([128, NB], F32R, tag="h1", bufs=2, name=f"h1{b}")
            evict(EVP["h1"][par], h1[:], h1p[:], bias=wf[:, _C_B1:_C_B1 + 1],
                  act=AF.Relu)
            yield

            h2ap = ppool.tile([128, NB], F32, tag="mlp", bufs=MLP_BUFS,
                              name=f"h2ap{b}")
            mm(h2ap[:], wr[:, _C_W2T:_C_W2T + 128], h1[:])
            yield
            h2a = wpool.tile([128, NB], F32R, tag="h2a", bufs=2,
                             name=f"h2a{b}")
            evict(EVP["h2a"][par], h2a[:], h2ap[:],
                  bias=wf[:, _C_B2A:_C_B2A + 1], act=AF.Relu)
            yield

            h2bp = ppool.tile([128, NB], F32, tag="mlp", bufs=MLP_BUFS,
                              name=f"h2bp{b}")
            mm(h2bp[:], wr[:, _C_W2T + 128:_C_W2T + 256], h1[:])
            yield
            h2b = wpool.tile([128, NB], F32R, tag="h2b", bufs=2,
                             name=f"h2b{b}")
            evict(EVP["h2b"][par], h2b[:], h2bp[:],
                  bias=wf[:, _C_B2B:_C_B2B + 1], act=AF.Relu)
            yield

            g1p = ppool.tile([64, NB], F32, tag="mlp", bufs=MLP_BUFS,
                             name=f"g1p{b}")
            mm(g1p[:], wr[:, _C_WW1T:_C_WW1T + 64], xt[:])
            yield
            g1 = wpool.tile([64, NB], F32R, tag="g1", bufs=2, name=f"g1{b}")
            evict(EVP["g1"][par], g1[:], g1p[:],
                  bias=wf[0:64, _C_WB1:_C_WB1 + 1], act=AF.Relu)
            yield

            g2p = ppool.tile([128, NB], F32, tag="mlp", bufs=MLP_BUFS,
                             name=f"g2p{b}")
            mm(g2p[:], wr[0:64, _C_WW2T:_C_WW2T + 128], g1[:])
            yield
            g2 = wpool.tile([128, NB], F32R, tag="g2", bufs=2, name=f"g2{b}")
            evict(EVP["g2"][par], g2[:], g2p[:],
                  bias=wf[:, _C_WB2:_C_WB2 + 1], act=AF.Relu)
            yield

            cpp = ppool.tile([20, NB], F32, tag="sm", bufs=SM_BUFS,
                             name=f"cpp{b}")
            mm(cpp[:], wr[:, _C_W3T:_C_W3T + 20], h2a[:], stop=False)
            mm(cpp[:], wr[:, _C_W3T + 20:_C_W3T + 40], h2b[:], start=False)
            yield
            cp = wpool.tile([20, NB], F32R, tag="cp", bufs=2, name=f"cp{b}")
            nc.scalar.activation(cp[:], cpp[:], AF.Tanh,
                                 bias=wf[0:20, _C_B3:_C_B3 + 1])
            yield

            wlp = ppool.tile([10, NB], F32, tag="sm", bufs=SM_BUFS,
                             name=f"wlp{b}")
            mm(wlp[:], wr[:, _C_WW3T:_C_WW3T + 10], g2[:])
            yield
            we2 = wpool.tile([10, NB], F32R, tag="we2", bufs=2,
                             name=f"we2{b}")
            nc.scalar.activation(we2[:], wlp[:], AF.Exp,
                                 bias=wf[0:10, _C_WB3:_C_WB3 + 1])
            yield

            pairp = ppool.tile([10, NB], F32, tag="sm", bufs=SM_BUFS,
                               name=f"pairp{b}")
            mm(pairp[:], wr[0:20, _C_P20:_C_P20 + 10], cp[:])
            yield
            we1 = wpool.tile([10, NB], F32R, tag="we1", bufs=2,
                             name=f"we1{b}")
            nc.vector.tensor_tensor(out=we1[:], in0=pairp[:],
                                    in1=we2[:].bitcast(F32), op=ALU.mult)
            yield

            # den first so num can reuse its sm slot right after the recip
            ndd = ppool.tile([64, NB], F32, tag="sm", bufs=SM_BUFS,
                             name=f"ndd{b}")
            mm(ndd[:], wr[0:10, _C_BTC:_C_BTC + S], we2[:])
            yield
            rec = wpool.tile([S, NB], F32, tag="rec", bufs=2, name=f"rec{b}")
            nc.vector.reciprocal_approx_fast(out=rec[:], in_=ndd[:])
            yield
            ndn = ppool.tile([64, NB], F32, tag="sm", bufs=SM_BUFS,
                             name=f"ndn{b}")
            mm(ndn[:], wr[0:10, _C_BTC:_C_BTC + S], we1[:])
            yield
            ratio = wpool.tile([S, NB], F32R, tag="ratio", bufs=2,
                               name=f"ratio{b}")
            nc.vector.tensor_tensor(out=ratio[:], in0=ndn[:],
                                    in1=rec[:], op=ALU.mult)
            ratios[b] = ratio
            pending.extend((b, jj) for jj in range(NB // 128))

        def drive_pair(b0, b1):
            gens = [stages(b0), stages(b1)]
            live = [True, True]
            i = 0
            since_tick = 0
            while live[0] or live[1]:
                g = gens[i % 2]
                if live[i % 2]:
                    try:
                        next(g)
                    except StopIteration:
                        live[i % 2] = False
                i += 1
                since_tick += 1
                if since_tick >= TICK_EVERY:
                    since_tick = 0
                    tick()

        drive_pair(0, 1)
        drive_pair(2, 3)
        while pending:
            emit_j(*pending.pop(0))

    nc.compile()
    return nc


def host_consts(cp_w1, cp_b1, cp_w2, cp_b2, cp_w3, cp_b3,
                w_w1, w_b1, w_w2, w_b2, w_w3, w_b3):
    wr = np.zeros((128, C_R), np.float32)
    wr[:, _C_W1T:_C_W1T + 128] = cp_w1.T       # [128,128]
    wr[:, _C_W2T:_C_W2T + 256] = cp_w2.T       # [128,256]
    w3t = cp_w3.T                              # [256,20]
    wr[:, _C_W3T:_C_W3T + 20] = w3t[0:128]
    wr[:, _C_W3T + 20:_C_W3T + 40] = w3t[128:256]
    wr[:, _C_WW1T:_C_WW1T + 64] = w_w1.T       # [128,64]
    wr[0:64, _C_WW2T:_C_WW2T + 128] = w_w2.T   # [64,128]
    wr[:, _C_WW3T:_C_WW3T + 10] = w_w3.T       # [128,10]
    p20 = np.zeros((20, 10), np.float32)
    for p in range(P):
        p20[2 * p, p] = 0.5
        p20[2 * p + 1, p] = 0.5
    wr[0:20, _C_P20:_C_P20 + 10] = p20
    wr[0:10, _C_BTC:_C_BTC + S] = basis_coarse()
    wr[:, _C_IDR:_C_IDR + 128] = np.eye(128, dtype=np.float32)
    wr = round_f32r(wr)

    im = round_f32r(interp_matrix())           # [S, F]

    wf = np.zeros((128, C_F), np.float32)
    wf[:, _C_B1] = cp_b1
    wf[:, _C_B2A] = cp_b2[0:128]
    wf[:, _C_B2B] = cp_b2[128:256]
    wf[0:20, _C_B3] = cp_b3
    wf[0:64, _C_WB1] = w_b1
    wf[:, _C_WB2] = w_b2
    wf[0:10, _C_WB3] = w_b3
    return wr, wf, im


_NC_CACHE = None


def get_program():
    global _NC_CACHE
    if _NC_CACHE is None:
        _NC_CACHE = build_program()
    return _NC_CACHE


def kernel(x, cp_w1, cp_b1, cp_w2, cp_b2, cp_w3, cp_b3,
           w_w1, w_b1, w_w2, w_b2, w_w3, w_b3, _return_raw=False):
    x = np.asarray(x, np.float32)
    wr, wf, im = host_consts(
        np.asarray(cp_w1, np.float32), np.asarray(cp_b1, np.float32),
        np.asarray(cp_w2, np.float32), np.asarray(cp_b2, np.float32),
        np.asarray(cp_w3, np.float32), np.asarray(cp_b3, np.float32),
        np.asarray(w_w1, np.float32), np.asarray(w_b1, np.float32),
        np.asarray(w_w2, np.float32), np.asarray(w_b2, np.float32),
        np.asarray(w_w3, np.float32), np.asarray(w_b3, np.float32))

    nc = get_program()
    in_maps = [
        {"x": np.ascontiguousarray(x[i * BC:(i + 1) * BC]),
         "wr": wr, "wf": wf, "im": im}
        for i in range(NCORES)
    ]
    res = run_bass_kernel_spmd(nc, in_maps, list(range(NCORES)))
    outs = [res.results[i]["out"] for i in range(NCORES)]
    full = np.concatenate(outs, axis=0)
    if _return_raw:
        return full, res
    return full


# revision 14
# speedup vs baseline: 1.0626x; 1.0626x over previous
"""Trainium2 Bass kernel for nn_CPWGenerator (B=16384, D=128, P=10, F=1024).

Data-parallel over batch across 8 NeuronCores (2048 rows/core). Per core:
  - feature-major 3-layer MLPs (control-point head + weight head)
  - softmax denominator cancels: out = (basis @ (e*cpm)) / (basis @ e)
  - KEY RESTRUCTURE vs the first version: the ratio num/den is evaluated at
    only S=64 coarse t-samples (basis row-normalization cancels in the
    ratio), then upsampled to F=1024 by a single PE matmul against a
    precomputed cubic-Lagrange interpolation matrix im [S, F]. The ratio is
    a sum of 10 Gaussians with sigma ~ 102 grid points, so cubic
    interpolation from 64 uniform samples adds < 2e-4 relative error
    (measured end-to-end: 5.2e-4 total vs reference, gate is 2e-2).
    This cuts the full-resolution elementwise work (reciprocal + multiply
    at [128, 1024] per tile) down to one divide at [64, 512] per block.
  - PSUM->SBUF evictions are spread across ACT, DVE and GPSIMD; output DMA
    is spread across the SP, ACT and GPSIMD queues.
Matmuls run as float32r (fp32 storage, 11-bit-mantissa operand rounding,
exact fp32 accumulation) at full PE rate.
"""
import sys
if "/opt/trn_rl_repo" not in sys.path:
    sys.path.insert(0, "/opt/trn_rl_repo")

from contextlib import ExitStack

import numpy as np

import concourse.bacc as bacc
import concourse.mybir as mybir
import concourse.tile as tile
from concourse.bass_utils import run_bass_kernel_spmd

F32 = mybir.dt.float32
F32R = mybir.dt.float32r
AF = mybir.ActivationFunctionType
ALU = mybir.AluOpType

# problem shapes (hardcoded per contest contract)
B, D, P, F = 16384, 128, 10, 1024
NCORES = 8
BC = B // NCORES          # rows per core = 2048
NB = 512                  # batch block
NBLK = BC // NB           # 4
S = 64                    # coarse t-samples for the ratio
EPS = 1e-8

# f32r const blob (wr) column offsets
_C_W1T = 0            # [128 x 128]
_C_W2T = 128          # [128 x 256]
_C_W3T = 384          # [128 x 40]  (W3Ta | W3Tb, 20 cols each)
_C_WW1T = 424         # [128 x 64]
_C_WW2T = 488         # [64  x 128]
_C_WW3T2 = 616        # [128 x 20]  w-head layer3 with pair-duplicated cols
_C_BTC = 636          # [20  x 64]  0.5 * basis at the S t-samples, pair rows
_C_IDR = 700          # [128 x 128] identity (for PE transpose)
C_R = 828

# fp32 bias blob (wf) columns
_C_B1 = 0
_C_B2A = 1
_C_B2B = 2
_C_B3 = 3             # rows 0-19
_C_WB1 = 4            # rows 0-63
_C_WB2 = 5
_C_WB3 = 6            # rows 0-19 (pair-duplicated)
C_F = 7

# ---- engine assignment tables (tunable) ----
# (GPSIMD cannot access PSUM — BIR verifier — so evictions are ACT/DVE only.)
# MLP eviction engine per (layer, block parity): 'A' = ACT, 'V' = DVE.
# Pair-lockstep runs blocks (2k, 2k+1) interleaved; same layer of the two
# blocks goes to opposite engines so the evicts run concurrently.
EVP = {"xt": "AV", "h1": "VA", "h2a": "AV", "h2b": "VA",
       "g1": "AV", "g2": "VA"}
# out-stage eviction engines per j-tile half (cols 0:512, 512:1024)
OEV = [("A", "V"), ("V", "A")] * 8
# out DMA queue per j-tile: 'S' = SP, 'A' = ACT, 'P' = GPSIMD
OQ = list("SPSS" "SPSP" "SPSS" "SPSP")
TICK_EVERY = 4            # stage-yields between injected out-tiles

MLP_BUFS = 2
SM_BUFS = 2
UP_BUFS = 4


def round_f32r(x: np.ndarray) -> np.ndarray:
    """fp32 -> fp32r rounding (keep 11 explicit mantissa bits, RNE)."""
    u = np.ascontiguousarray(x, dtype=np.float32).view(np.uint32)
    keep = np.uint32(0xFFFFF000)
    half = np.uint32(0x800)
    lsb = (u >> np.uint32(12)) & np.uint32(1)
    r = (u + half - np.uint32(1) + lsb) & keep
    return r.view(np.float32)


def basis_coarse() -> np.ndarray:
    """Unnormalized Gaussian basis at S uniform t-samples: [P, S].
    Row-normalization of the reference basis cancels in num/den."""
    t = np.linspace(0.0, 1.0, S, dtype=np.float64)
    centers = np.arange(P, dtype=np.float64) / (P - 1)
    sigma = 1.0 / P
    z = t[None, :] - centers[:, None]
    return np.exp(-(z * z) / (2.0 * sigma * sigma)).astype(np.float32)


def interp_matrix() -> np.ndarray:
    """Cubic Lagrange interpolation weights from S uniform t-samples to the
    F output t-points: [S, F]."""
    t_s = np.linspace(0.0, 1.0, S)
    t_f = np.linspace(0.0, 1.0, F)
    I = np.zeros((S, F), np.float64)
    for fi, t in enumerate(t_f):
        k = np.searchsorted(t_s, t) - 1
        k = int(np.clip(k, 1, S - 3))
        idx = [k - 1, k, k + 1, k + 2]
        for a in range(4):
            w = 1.0
            for bb in range(4):
                if a != bb:
                    w *= (t - t_s[idx[bb]]) / (t_s[idx[a]] - t_s[idx[bb]])
            I[idx[a], fi] += w
    return I.astype(np.float32)


def build_program():
    nc = bacc.Bacc()
    x_in = nc.declare_dram_parameter("x", [BC, D], F32R, isOutput=False)
    wr_in = nc.declare_dram_parameter("wr", [128, C_R], F32R, isOutput=False)
    wf_in = nc.declare_dram_parameter("wf", [128, C_F], F32, isOutput=False)
    im_in = nc.declare_dram_parameter("im", [S, F], F32R, isOutput=False)
    out = nc.declare_dram_parameter("out", [BC, F], F32, isOutput=True)

    with tile.TileContext(nc) as tc, ExitStack() as ctx:
        cpool = ctx.enter_context(tc.tile_pool(name="const", bufs=1))
        wpool = ctx.enter_context(tc.tile_pool(name="work", bufs=2))
        opool = ctx.enter_context(tc.tile_pool(name="outp", bufs=4))
        ppool = ctx.enter_context(tc.tile_pool(name="ps", bufs=1, space="PSUM"))

        wr = cpool.tile([128, C_R], F32R)
        wf = cpool.tile([128, C_F], F32)
        im = cpool.tile([S, F], F32R)
        xall = cpool.tile([128, BC], F32R)

        def x_dma(eng, r0, r1):
            eng.dma_start(
                xall[:, r0:r1].rearrange("p (c d) -> p c d", c=(r1 - r0) // 128),
                x_in[r0:r1, :].rearrange("(c p) d -> p c d", p=128),
            )

        # fill-phase DMAs, ordered by first use; ACT stays DMA-free (it is an
        # eviction bottleneck engine). SP: ident -> x blocks -> interp matrix.
        # Pool: biases -> weights -> x blocks.
        x_dma(nc.sync, 0, 512)
        nc.gpsimd.dma_start(wr[:, _C_IDR:_C_IDR + 128],
                            wr_in[:, _C_IDR:_C_IDR + 128])
        nc.gpsimd.dma_start(wf[:], wf_in[:])
        nc.gpsimd.dma_start(wr[:, 0:_C_IDR], wr_in[:, 0:_C_IDR])
        x_dma(nc.sync, 512, 1024)
        x_dma(nc.sync, 1024, 1536)
        nc.sync.dma_start(im[:], im_in[:])
        x_dma(nc.gpsimd, 1536, 2048)

        def mm(out_ap, lhsT, rhs, start=True, stop=True):
            nc.tensor.matmul(out_ap, lhsT, rhs, start=start, stop=stop)

        def evict(eng, out_ap, in_ap, bias=None, act=None):
            """PSUM -> SBUF eviction with optional bias+activation."""
            if eng == "A":
                if act is None:
                    nc.scalar.activation(out_ap, in_ap, AF.Copy)
                else:
                    nc.scalar.activation(out_ap, in_ap, act, bias=bias)
            else:
                e = nc.vector if eng == "V" else nc.gpsimd
                if act is None and bias is None:
                    e.tensor_scalar(out=out_ap, in0=in_ap, scalar1=0.0,
                                    scalar2=None, op0=ALU.add)
                elif act == AF.Relu:
                    e.tensor_scalar(out=out_ap, in0=in_ap, scalar1=bias,
                                    scalar2=0.0, op0=ALU.add, op1=ALU.max)
                else:
                    raise ValueError((eng, act))

        ratios = [None] * NBLK
        pending = []          # j-tiles awaiting emission (ready after ratio)

        def emit_j(b, jj):
            x0 = NB * b
            ratio = ratios[b]
            j = (NB // 128) * b + jj
            lhsT = ratio[:, 128 * jj:128 * (jj + 1)]
            upa = ppool.tile([128, 512], F32, tag="up", bufs=UP_BUFS,
                             name=f"upa{j}")
            mm(upa[:], lhsT, im[:, 0:512])
            upb = ppool.tile([128, 512], F32, tag="up", bufs=UP_BUFS,
                             name=f"upb{j}")
            mm(upb[:], lhsT, im[:, 512:1024])
            o = opool.tile([128, F], F32, tag="o", name=f"o{j}")
            evict(OEV[j][0], o[:, 0:512], upa[:])
            evict(OEV[j][1], o[:, 512:1024], upb[:])
            q = {"S": nc.sync, "A": nc.scalar, "P": nc.gpsimd}[OQ[j]]
            q.dma_start(out[x0 + 128 * jj:x0 + 128 * (jj + 1), :], o[:])

        def tick():
            if pending:
                emit_j(*pending.pop(0))

        def stages(b):
            """Generator emitting one block's ops, yielding between stages so
            the driver can interleave the pair partner + out-tiles."""
            par = b % 2
            x0 = NB * b

            xtp = ppool.tile([128, NB], F32R, tag="mlp", bufs=MLP_BUFS,
                             name=f"xtp{b}")
            for c in range(NB // 128):
                nc.tensor.matmul(
                    xtp[:, 128 * c:128 * (c + 1)],
                    xall[:, x0 + 128 * c:x0 + 128 * (c + 1)],
                    wr[:, _C_IDR:_C_IDR + 128],
                    is_transpose=True,
                    start=(c == 0), stop=(c == NB // 128 - 1),
                )
            yield
            xt = wpool.tile([128, NB], F32R, tag="xt", bufs=2, name=f"xt{b}")
            evict(EVP["xt"][par], xt[:], xtp[:].bitcast(F32))
            yield

            h1p = ppool.tile([128, NB], F32, tag="mlp", bufs=MLP_BUFS,
                             name=f"h1p{b}")
            mm(h1p[:], wr[:, _C_W1T:_C_W1T + 128], xt[:])
            yield
            h1 = wpool.tile([128, NB], F32R, tag="h1", bufs=2, name=f"h1{b}")
            evict(EVP["h1"][par], h1[:], h1p[:], bias=wf[:, _C_B1:_C_B1 + 1],
                  act=AF.Relu)
            yield

            h2ap = ppool.tile([128, NB], F32, tag="mlp", bufs=MLP_BUFS,
                              name=f"h2ap{b}")
            mm(h2ap[:], wr[:, _C_W2T:_C_W2T + 128], h1[:])
            yield
            h2a = wpool.tile([128, NB], F32R, tag="h2a", bufs=2,
                             name=f"h2a{b}")
            evict(EVP["h2a"][par], h2a[:], h2ap[:],
                  bias=wf[:, _C_B2A:_C_B2A + 1], act=AF.Relu)
            yield

            h2bp = ppool.tile([128, NB], F32, tag="mlp", bufs=MLP_BUFS,
                              name=f"h2bp{b}")
            mm(h2bp[:], wr[:, _C_W2T + 128:_C_W2T + 256], h1[:])
            yield
            h2b = wpool.tile([128, NB], F32R, tag="h2b", bufs=2,
                             name=f"h2b{b}")
            evict(EVP["h2b"][par], h2b[:], h2bp[:],
                  bias=wf[:, _C_B2B:_C_B2B + 1], act=AF.Relu)
            yield

            g1p = ppool.tile([64, NB], F32, tag="mlp", bufs=MLP_BUFS,
                             name=f"g1p{b}")
            mm(g1p[:], wr[:, _C_WW1T:_C_WW1T + 64], xt[:])
            yield
            g1 = wpool.tile([64, NB], F32R, tag="g1", bufs=2, name=f"g1{b}")
            evict(EVP["g1"][par], g1[:], g1p[:],
                  bias=wf[0:64, _C_WB1:_C_WB1 + 1], act=AF.Relu)
            yield

            g2p = ppool.tile([128, NB], F32, tag="mlp", bufs=MLP_BUFS,
                             name=f"g2p{b}")
            mm(g2p[:], wr[0:64, _C_WW2T:_C_WW2T + 128], g1[:])
            yield
            g2 = wpool.tile([128, NB], F32R, tag="g2", bufs=2, name=f"g2{b}")
            evict(EVP["g2"][par], g2[:], g2p[:],
                  bias=wf[:, _C_WB2:_C_WB2 + 1], act=AF.Relu)
            yield

            cpp = ppool.tile([20, NB], F32, tag="sm", bufs=SM_BUFS,
                             name=f"cpp{b}")
            mm(cpp[:], wr[:, _C_W3T:_C_W3T + 20], h2a[:], stop=False)
            mm(cpp[:], wr[:, _C_W3T + 20:_C_W3T + 40], h2b[:], start=False)
            yield
            cp = wpool.tile([20, NB], F32R, tag="cp", bufs=2, name=f"cp{b}")
            nc.scalar.activation(cp[:], cpp[:], AF.Tanh,
                                 bias=wf[0:20, _C_B3:_C_B3 + 1])
            yield

            wlp = ppool.tile([20, NB], F32, tag="sm", bufs=SM_BUFS,
                             name=f"wlp{b}")
            mm(wlp[:], wr[:, _C_WW3T2:_C_WW3T2 + 20], g2[:])
            yield
            we2 = wpool.tile([20, NB], F32R, tag="we2", bufs=2,
                             name=f"we2{b}")
            nc.scalar.activation(we2[:], wlp[:], AF.Exp,
                                 bias=wf[0:20, _C_WB3:_C_WB3 + 1])
            yield
            # wcpm = cp * e2: SBUF x SBUF, runs on GPSIMD (no PSUM access)
            we1 = wpool.tile([20, NB], F32R, tag="we1", bufs=2,
                             name=f"we1{b}")
            nc.gpsimd.tensor_tensor(out=we1[:], in0=cp[:].bitcast(F32),
                                    in1=we2[:].bitcast(F32), op=ALU.mult)
            yield

            # den first so num can reuse its sm slot right after the recip
            ndd = ppool.tile([64, NB], F32, tag="sm", bufs=SM_BUFS,
                             name=f"ndd{b}")
            mm(ndd[:], wr[0:20, _C_BTC:_C_BTC + S], we2[:])
            yield
            rec = wpool.tile([S, NB], F32, tag="rec", bufs=2, name=f"rec{b}")
            nc.vector.reciprocal_approx_fast(out=rec[:], in_=ndd[:])
            yield
            ndn = ppool.tile([64, NB], F32, tag="sm", bufs=SM_BUFS,
                             name=f"ndn{b}")
            mm(ndn[:], wr[0:20, _C_BTC:_C_BTC + S], we1[:])
            yield
            ratio = wpool.tile([S, NB], F32R, tag="ratio", bufs=2,
                               name=f"ratio{b}")
            nc.vector.tensor_tensor(out=ratio[:], in0=ndn[:],
                                    in1=rec[:], op=ALU.mult)
            ratios[b] = ratio
            pending.extend((b, jj) for jj in range(NB // 128))

        # sliding window of 2 concurrently-emitted blocks: when one block's
        # stage stream ends, the next block takes its slot immediately.
        window = [stages(0), stages(1)]
        nxt = 2
        i = 0
        since_tick = 0
        while window:
            g = window[i % len(window)]
            try:
                next(g)
                i += 1
            except StopIteration:
                if nxt < NBLK:
                    window[window.index(g)] = stages(nxt)
                    nxt += 1
                else:
                    window.remove(g)
            since_tick += 1
            if since_tick >= TICK_EVERY:
                since_tick = 0
                tick()
        while pending:
            emit_j(*pending.pop(0))

    nc.compile()
    return nc


def host_consts(cp_w1, cp_b1, cp_w2, cp_b2, cp_w3, cp_b3,
                w_w1, w_b1, w_w2, w_b2, w_w3, w_b3):
    wr = np.zeros((128, C_R), np.float32)
    wr[:, _C_W1T:_C_W1T + 128] = cp_w1.T       # [128,128]
    wr[:, _C_W2T:_C_W2T + 256] = cp_w2.T       # [128,256]
    w3t = cp_w3.T                              # [256,20]
    wr[:, _C_W3T:_C_W3T + 20] = w3t[0:128]
    wr[:, _C_W3T + 20:_C_W3T + 40] = w3t[128:256]
    wr[:, _C_WW1T:_C_WW1T + 64] = w_w1.T       # [128,64]
    wr[0:64, _C_WW2T:_C_WW2T + 128] = w_w2.T   # [64,128]
    w3t2 = np.repeat(w_w3.T, 2, axis=1)        # [128,20] pair-duplicated
    wr[:, _C_WW3T2:_C_WW3T2 + 20] = w3t2
    wr[0:20, _C_BTC:_C_BTC + S] = 0.5 * np.repeat(basis_coarse(), 2, axis=0)
    wr[:, _C_IDR:_C_IDR + 128] = np.eye(128, dtype=np.float32)
    wr = round_f32r(wr)

    im = round_f32r(interp_matrix())           # [S, F]

    wf = np.zeros((128, C_F), np.float32)
    wf[:, _C_B1] = cp_b1
    wf[:, _C_B2A] = cp_b2[0:128]
    wf[:, _C_B2B] = cp_b2[128:256]
    wf[0:20, _C_B3] = cp_b3
    wf[0:64, _C_WB1] = w_b1
    wf[:, _C_WB2] = w_b2
    wf[0:20, _C_WB3] = np.repeat(w_b3, 2)
    return wr, wf, im


_NC_CACHE = None


def get_program():
    global _NC_CACHE
    if _NC_CACHE is None:
        _NC_CACHE = build_program()
    return _NC_CACHE


def kernel(x, cp_w1, cp_b1, cp_w2, cp_b2, cp_w3, cp_b3,
           w_w1, w_b1, w_w2, w_b2, w_w3, w_b3, _return_raw=False):
    x = np.asarray(x, np.float32)
    wr, wf, im = host_consts(
        np.asarray(cp_w1, np.float32), np.asarray(cp_b1, np.float32),
        np.asarray(cp_w2, np.float32), np.asarray(cp_b2, np.float32),
        np.asarray(cp_w3, np.float32), np.asarray(cp_b3, np.float32),
        np.asarray(w_w1, np.float32), np.asarray(w_b1, np.float32),
        np.asarray(w_w2, np.float32), np.asarray(w_b2, np.float32),
        np.asarray(w_w3, np.float32), np.asarray(w_b3, np.float32))

    nc = get_program()
    in_maps = [
        {"x": np.ascontiguousarray(x[i * BC:(i + 1) * BC]),
         "wr": wr, "wf": wf, "im": im}
        for i in range(NCORES)
    ]
    res = run_bass_kernel_spmd(nc, in_maps, list(range(NCORES)))
    outs = [res.results[i]["out"] for i in range(NCORES)]
    full = np.concatenate(outs, axis=0)
    if _return_raw:
        return full, res
    return full
